# revision 8
# baseline (speedup 1.0000x reference)
"""GatedLinearRecurrence Trainium2 kernel (8-core SPMD, Bass/Tile).

Strategy: shard (batch=2) x (4 sequence chunks of 1024 tokens) across 8 cores.
Each core processes 1152 tokens: a 128-token warm-up window (recomputed
redundantly; the recurrence decay makes carry-in truncation error ~1e-24)
followed by its 1024 "main" tokens.  No collectives needed.

v2 changes vs baseline:
  * bf16 operands for in_proj / out_proj / transposes (same PE rate as f32r
    at these widths, but half the DMA + SBUF, 1.0 c/row transposes).
  * gate matmul in fp8 e4m3 with MatmulPerfMode.DoubleRow (K=256 per
    instruction, 2x PE throughput).  gw is pre-scaled x32 and xc x4 on the
    fp8 copy; the 1/128 is folded into the sigmoid evacuation scale.
    Simulated end-to-end rel err 0.0049 (gate 2e-2).
  * norm_b folded into a per-output-channel in_proj bias (host-computed
    c = in_proj_w @ norm_b), so transposed x-hat needs no bias and all 8
    d-tiles of a token tile evacuate PSUM in ONE scalar op.
  * z kept in SBUF as silu(z) bf16 (no HBM scratch roundtrip).
  * phase order S2(in_proj-x) -> S3'(gate et interleaved with z et) -> S7
    (out_proj): the PE never waits on the DVE scans, z fills the gaps.
  * scans/bt/yz alternate vector/gpsimd by et parity; conv split across
    vector/gpsimd; out_proj final pass runs kt-inner so the 8 tail
    evacuations stagger instead of serializing.
"""
import sys

for p in ("/opt/trn_rl_repo", "/root/.axon_site/_ro/trn_rl_repo"):
    if p not in sys.path:
        sys.path.insert(0, p)

import numpy as np
import ml_dtypes

import concourse.bass as bass
import concourse.bacc as bacc
import concourse.tile as tile
import concourse.mybir as mybir
from concourse.bass_utils import run_bass_kernel_spmd
from concourse.masks import make_identity

F32 = mybir.dt.float32
BF16 = mybir.dt.bfloat16
FP8 = mybir.dt.float8e4
AF = mybir.ActivationFunctionType
OP = mybir.AluOpType
DR = mybir.MatmulPerfMode.DoubleRow

B, L, D = 2, 4096, 1024
DI = 2048            # d_inner
NT = 1152            # tokens per core (128 warm-up + 1024 main)
W = 128              # warm-up tokens
CHUNK = 1024
NTT = NT // 128      # 9 token tiles
KD = D // 128        # 8 k-tiles over d_model
KC = DI // 128       # 16 k-tiles over d_inner
KC2 = KC // 2        # 8 fp8 k-pair tiles (DoubleRow contracts 256)
TC = 384             # matmul N chunk (3 per core)
NTC = NT // TC
EPS = 1e-5
SW = 32.0            # gate weight fp8 pre-scale
SX = 4.0             # xc fp8 pre-scale

_cache = {}


def _build():
    nc = bacc.Bacc(None, target_bir_lowering=False)

    x_h = nc.dram_tensor("x", [NT, D], F32, kind="ExternalInput")
    w1x_h = nc.dram_tensor("w1x", [D, DI], BF16, kind="ExternalInput")
    w1z_h = nc.dram_tensor("w1z", [D, DI], BF16, kind="ExternalInput")
    gw8_h = nc.dram_tensor("gw8", [KC, 128, KC2 * 2 * 128], FP8, kind="ExternalInput")
    op_h = nc.dram_tensor("opw", [DI, D], BF16, kind="ExternalInput")
    convw_h = nc.dram_tensor("convw", [128, KC * 4], F32, kind="ExternalInput")
    convb_h = nc.dram_tensor("convb", [128, KC], F32, kind="ExternalInput")
    gateb_h = nc.dram_tensor("gateb", [128, KC], F32, kind="ExternalInput")
    cx_h = nc.dram_tensor("cx", [128, KC], F32, kind="ExternalInput")
    cz_h = nc.dram_tensor("cz", [128, KC], F32, kind="ExternalInput")
    mask_h = nc.dram_tensor("mask", [1, W], BF16, kind="ExternalInput")
    out_h = nc.dram_tensor("out", [CHUNK, D], F32, kind="ExternalOutput")

    with tile.TileContext(nc) as tc:
        with tc.tile_pool(name="consts", bufs=1) as consts:

            ident = consts.tile([128, 128], BF16, name="ident")
            make_identity(nc, ident)
            mask_sb = consts.tile([128, W], BF16, name="mask_sb")
            nc.gpsimd.dma_start(
                out=mask_sb,
                in_=bass.AP(tensor=mask_h, offset=0, ap=[[0, 128], [1, W]]))
            convw = consts.tile([128, KC * 4], F32, name="convw")
            nc.gpsimd.dma_start(out=convw, in_=convw_h.ap())
            convb = consts.tile([128, KC], F32, name="convb")
            nc.gpsimd.dma_start(out=convb, in_=convb_h.ap())
            gateb = consts.tile([128, KC], F32, name="gateb")
            nc.gpsimd.dma_start(out=gateb, in_=gateb_h.ap())
            cxb = consts.tile([128, KC], F32, name="cxb")
            nc.gpsimd.dma_start(out=cxb, in_=cx_h.ap())
            czb = consts.tile([128, KC], F32, name="czb")
            nc.gpsimd.dma_start(out=czb, in_=cz_h.ap())
            eps_t = consts.tile([128, 1], F32, name="eps_t")
            nc.vector.memset(eps_t, EPS)

            # long-lived activation stores
            with tc.tile_pool(name="xcp", bufs=1) as xcp, \
                 tc.tile_pool(name="xc8p", bufs=1) as xc8p, \
                 tc.tile_pool(name="gws", bufs=4) as gs, \
                 tc.tile_pool(name="xT", bufs=1) as xTp:

                xc = [xcp.tile([128, NT], BF16, name=f"xct{e}") for e in range(KC)]
                xc8 = [xc8p.tile([128, 2, NT], FP8, name=f"xc8t{j}")
                       for j in range(KC2)]
                # x-hat-T per chunk: [128 d-part, KD d-tiles, TC tokens]
                xT = [xTp.tile([128, KD, TC], BF16, name=f"xTt{c_}")
                      for c_ in range(NTC)]

                # prefetch first gate-weight tiles early (gpsimd queue head)
                gts = {}
                for et in range(2):
                    gt = gs.tile([128, KC2, 2, 128], FP8, tag="gw", name=f"gt{et}")
                    nc.gpsimd.dma_start(out=gt, in_=gw8_h.ap()[et])
                    gts[et] = gt

                # ---- S1-S2: LN, transpose, in_proj x-half, conv, silu ----
                with tc.tile_pool(name="s1roll", bufs=2) as s1r, \
                     tc.tile_pool(name="stat", bufs=4) as stp, \
                     tc.tile_pool(name="w1s", bufs=3) as ws, \
                     tc.tile_pool(name="psmm", bufs=4, space="PSUM") as psmm, \
                     tc.tile_pool(name="pstr", bufs=3, space="PSUM") as pstr:

                    for it in range(NTT):
                        tc3, col = it // 3, (it % 3) * 128
                        xt = s1r.tile([128, D], F32, tag="xt", bufs=4, name="xt")
                        eng = nc.sync if it % 2 == 0 else nc.scalar
                        eng.dma_start(out=xt, in_=x_h.ap()[it * 128:(it + 1) * 128, :])
                        stats = stp.tile([128, 2, 6], F32, tag="stats", name="stats")
                        nc.vector.bn_stats(out=stats[:, 0, :], in_=xt[:, 0:512])
                        nc.vector.bn_stats(out=stats[:, 1, :], in_=xt[:, 512:1024])
                        mv = stp.tile([128, 2], F32, tag="mv", name="mv")
                        nc.vector.bn_aggr(out=mv, in_=stats)
                        rstd = stp.tile([128, 1], F32, tag="rstd", name="rstd")
                        nc.scalar.activation(out=rstd, in_=mv[:, 1:2], func=AF.Sqrt,
                                             bias=eps_t, scale=1.0)
                        nc.vector.reciprocal(out=rstd, in_=rstd)
                        nmr = stp.tile([128, 1], F32, tag="nmr", name="nmr")
                        nc.vector.tensor_scalar(out=nmr, in0=mv[:, 0:1],
                                                scalar1=rstd, scalar2=-1.0,
                                                op0=OP.mult, op1=OP.mult)
                        xh = s1r.tile([128, D], BF16, tag="xh", bufs=3, name="xh")
                        # LN apply on the Scalar engine: x*rstd - mu*rstd
                        nc.scalar.activation(out=xh, in_=xt, func=AF.Identity,
                                             scale=rstd, bias=nmr)
                        pst = pstr.tile([128, KD, 128], BF16, tag="tr", name="pst")
                        # one accumulation group over the 8 disjoint d-tile
                        # regions: first write after start zeroes per-byte
                        for d_ in range(KD):
                            nc.tensor.matmul(pst[:, d_, :],
                                             xh[:, d_ * 128:(d_ + 1) * 128], ident,
                                             start=(d_ == 0), stop=(d_ == KD - 1),
                                             is_transpose=True,
                                             skip_group_check=True)
                        # one evacuation for all 8 d-tiles of this token tile
                        nc.scalar.activation(
                            out=xT[tc3][:, :, col:col + 128], in_=pst,
                            func=AF.Identity, scale=1.0)

                    # in_proj x-half + conv + silu + warm-up mask + fp8 cast.
                    NW = 4
                    order = [(e, c) for c in range(NTC) for e in range(NW)]
                    order += [(e, c) for e in range(NW, KC) for c in range(NTC)]
                    wts, xins = {}, {}

                    def s2_chain(et, tc3):
                        if tc3 == 0:
                            wt = ws.tile([128, KD, 128], BF16, tag="w1",
                                         bufs=6, name=f"wt{et}")
                            nc.sync.dma_start(
                                out=wt,
                                in_=w1x_h.ap()[:, et * 128:(et + 1) * 128]
                                .rearrange("(kt p) e -> p kt e", p=128))
                            wts[et] = wt
                            xin = s1r.tile([128, NT + 3], BF16, tag="xin",
                                           bufs=NW + 2, name=f"xin{et}")
                            nc.vector.memset(xin[:, 0:3], 0.0)
                            xins[et] = xin
                        ps = psmm.tile([128, TC], F32, tag="mm", name="ps")
                        for kt in range(KD):
                            nc.tensor.matmul(
                                ps, wts[et][:, kt, :], xT[tc3][:, kt, :],
                                start=(kt == 0), stop=(kt == KD - 1))
                        nc.scalar.activation(
                            out=xins[et][:, 3 + tc3 * TC: 3 + (tc3 + 1) * TC],
                            in_=ps, func=AF.Identity, bias=cxb[:, et:et + 1],
                            scale=1.0)
                        if tc3 == NTC - 1:
                            xin = xins.pop(et)
                            tmp = s1r.tile([128, NT], BF16, tag="ctmp", bufs=3,
                                           name="ctmp")
                            nc.vector.tensor_scalar_mul(
                                tmp, xin[:, 0:NT], convw[:, et * 4:et * 4 + 1])
                            for k in range(1, 4):
                                nc.vector.scalar_tensor_tensor(
                                    out=tmp, in0=xin[:, k:k + NT],
                                    scalar=convw[:, et * 4 + k:et * 4 + k + 1],
                                    in1=tmp, op0=OP.mult, op1=OP.add)
                            nc.scalar.activation(out=xc[et], in_=tmp, func=AF.Silu,
                                                 bias=convb[:, et:et + 1], scale=1.0)
                            # mask is non-unit only on the warm-up columns
                            nc.vector.tensor_mul(
                                xc[et][:, 0:W], xc[et][:, 0:W], mask_sb)
                            # fp8 copy (x SX) for the gate matmul rhs
                            nc.gpsimd.tensor_scalar_mul(
                                xc8[et // 2][:, et % 2, :], xc[et], SX)

                    for et, tc3 in order:
                        s2_chain(et, tc3)

                # ---- S3': per et: gate (fp8 DoubleRow) + z-half (bf16),
                # sigmoid/scan/yz trail on scalar + vector/gpsimd. ----
                with tc.tile_pool(name="yp", bufs=1) as yp:
                    ych = [[None] * NTC for _ in range(KC)]
                    with tc.tile_pool(name="w1zs", bufs=3) as wzs, \
                         tc.tile_pool(name="zsil", bufs=4) as zsp, \
                         tc.tile_pool(name="ach", bufs=6) as ayp, \
                         tc.tile_pool(name="btr", bufs=4) as btp, \
                         tc.tile_pool(name="psg", bufs=2, space="PSUM") as psg, \
                         tc.tile_pool(name="psz", bufs=2, space="PSUM") as psz:

                        zs = {}

                        def z_half(et):
                            wt = wzs.tile([128, KD, 128], BF16, tag="wz", name="wtz")
                            nc.sync.dma_start(
                                out=wt,
                                in_=w1z_h.ap()[:, et * 128:(et + 1) * 128]
                                .rearrange("(kt p) e -> p kt e", p=128))
                            zt = zsp.tile([128, CHUNK], BF16, tag="zs", name="zst")
                            zs[et] = zt
                            for tc3 in range(NTC):
                                lo = W if tc3 == 0 else tc3 * TC   # NT-space
                                n = (tc3 + 1) * TC - lo
                                ps = psz.tile([128, TC], F32, tag="zmm", name="pszt")
                                for kt in range(KD):
                                    nc.tensor.matmul(
                                        ps[:, 0:n], wt[:, kt, :],
                                        xT[tc3][:, kt, TC - n:TC],
                                        start=(kt == 0), stop=(kt == KD - 1))
                                nc.scalar.activation(
                                    out=zt[:, lo - W:(tc3 + 1) * TC - W],
                                    in_=ps[:, 0:n], func=AF.Silu,
                                    bias=czb[:, et:et + 1], scale=1.0)

                        def gate(et):
                            gt = gts.pop(et)
                            if et + 2 < KC:
                                g2 = gs.tile([128, KC2, 2, 128], FP8, tag="gw",
                                             name=f"gt{et + 2}")
                                nc.gpsimd.dma_start(out=g2, in_=gw8_h.ap()[et + 2])
                                gts[et + 2] = g2
                            seng = nc.vector
                            prev_y = None
                            for tc3 in range(NTC):
                                a_t = ayp.tile([128, TC], BF16, tag="ach", name="ach")
                                ps = psg.tile([128, TC], F32, tag="gmm", name="psgt")
                                for j in range(KC2):
                                    nc.tensor.matmul(
                                        ps, gt[:, j], xc8[j][:, :, tc3 * TC:(tc3 + 1) * TC],
                                        start=(j == 0), stop=(j == KC2 - 1),
                                        perf_mode=DR)
                                nc.scalar.activation(
                                    out=a_t, in_=ps, func=AF.Sigmoid,
                                    bias=gateb[:, et:et + 1], scale=1.0 / (SW * SX))
                                bt = btp.tile([128, TC], BF16, tag="bt", name="bt")
                                seng.scalar_tensor_tensor(
                                    out=bt, in0=a_t, scalar=1.0,
                                    in1=xc[et][:, tc3 * TC:(tc3 + 1) * TC],
                                    op0=OP.subtract, op1=OP.mult)
                                y_t = yp.tile([128, TC], BF16, name=f"y{et}_{tc3}")
                                init = 0.0 if tc3 == 0 else prev_y[:, TC - 1:TC]
                                seng.tensor_tensor_scan(
                                    out=y_t, data0=a_t, data1=bt, initial=init,
                                    op0=OP.mult, op1=OP.add)
                                ych[et][tc3] = y_t
                                prev_y = y_t
                            # -yg: multiply once the carry chain is complete
                            zt = zs.pop(et)
                            for tc3 in range(NTC):
                                lo = max(tc3 * TC, W) - tc3 * TC
                                seng.tensor_mul(
                                    ych[et][tc3][:, lo:TC],
                                    ych[et][tc3][:, lo:TC],
                                    zt[:, tc3 * TC + lo - W:(tc3 + 1) * TC - W])

                        z_half(0)
                        z_half(1)
                        for et in range(KC):
                            gate(et)
                            if et + 2 < KC:
                                z_half(et + 2)

                    # ---- S7: out_proj + residual. ----
                    NTB = CHUNK // 128

                    def yslice(kt, tb):
                        col = W + tb * 128          # absolute column in [0, NT)
                        tc3, off = col // TC, col % TC
                        return ych[kt][tc3][:, off:off + 128]

                    with tc.tile_pool(name="ops", bufs=18) as opp, \
                         tc.tile_pool(name="s7roll", bufs=6) as s7r, \
                         tc.tile_pool(name="s7res", bufs=8) as s7x, \
                         tc.tile_pool(name="psop", bufs=8, space="PSUM") as psop:
                        for nb in range(2):
                            xres = {}
                            for tb in range(NTB):
                                xres[tb] = s7x.tile([128, 512], F32, tag="xres",
                                                    name=f"xres{tb}")
                                nc.scalar.dma_start(
                                    out=xres[tb],
                                    in_=x_h.ap()[W + tb * 128:W + (tb + 1) * 128,
                                                 nb * 512:(nb + 1) * 512])
                            pss = [psop.tile([128, 512], F32, tag="op",
                                             name=f"pso{tb}") for tb in range(NTB)]
                            opts = []
                            for kt in range(KC):
                                opt = opp.tile([128, 512], BF16, tag="opw",
                                               name="opt")
                                nc.gpsimd.dma_start(
                                    out=opt,
                                    in_=op_h.ap()[kt * 128:(kt + 1) * 128,
                                                  nb * 512:(nb + 1) * 512])
                                opts.append(opt)
                            if nb == 0:
                                # kt-outer: max stationary reuse
                                for kt in range(KC):
                                    for tb in range(NTB):
                                        nc.tensor.matmul(
                                            pss[tb], yslice(kt, tb), opts[kt],
                                            start=(kt == 0), stop=(kt == KC - 1))
                            else:
                                # kt-inner: stagger the final evacuations
                                for tb in range(NTB):
                                    for kt in range(KC):
                                        nc.tensor.matmul(
                                            pss[tb], yslice(kt, tb), opts[kt],
                                            start=(kt == 0), stop=(kt == KC - 1))
                            for tb in range(NTB):
                                oh = s7r.tile([128, 512], F32, tag="oh", name="oh")
                                nc.vector.tensor_sub(oh, xres[tb], pss[tb])
                                deng = nc.sync if tb % 2 == 0 else nc.scalar
                                deng.dma_start(
                                    out=out_h.ap()[tb * 128:(tb + 1) * 128,
                                                   nb * 512:(nb + 1) * 512],
                                    in_=oh)

    nc.compile()
    return nc


def _prep_host(x, norm_w, norm_b, in_proj_w, conv_w, conv_b, gate_w, gate_b,
               out_proj_w):
    bf16 = ml_dtypes.bfloat16
    fp8 = ml_dtypes.float8_e4m3
    w1f = (in_proj_w * norm_w[None, :]).astype(np.float32)
    cbias = (in_proj_w.astype(np.float64) @ norm_b.astype(np.float64)).astype(np.float32)
    w1xT = np.ascontiguousarray(w1f[:DI].T).astype(bf16)      # [D, DI]
    w1zT = np.ascontiguousarray(w1f[DI:].T).astype(bf16)      # [D, DI]
    # gw8[et, p, j, s, m] = gw[et*128+m, j*256+s*128+p] * SW
    gwT = np.ascontiguousarray(gate_w.T * SW)                 # [c_in, e_out]
    gw8 = gwT.reshape(KC2, 2, 128, KC, 128).transpose(3, 2, 0, 1, 4)
    gw8 = np.ascontiguousarray(gw8.reshape(KC, 128, KC2 * 2 * 128)).astype(fp8)
    opT = np.ascontiguousarray(out_proj_w.T).astype(bf16)     # [DI, D]
    convw_r = np.ascontiguousarray(
        conv_w.reshape(KC, 128, 4).transpose(1, 0, 2).reshape(128, KC * 4))
    convb_r = np.ascontiguousarray(conv_b.reshape(KC, 128).T)
    gateb_r = np.ascontiguousarray(gate_b.reshape(KC, 128).T)
    cx_r = np.ascontiguousarray(cbias[:DI].reshape(KC, 128).T)
    cz_r = np.ascontiguousarray(cbias[DI:].reshape(KC, 128).T)

    in_maps = []
    for core in range(8):
        b, j = core // 4, core % 4
        xs = np.zeros((NT, D), np.float32)
        start = j * CHUNK - W
        mask = np.ones((1, W), np.float32)
        if j == 0:
            xs[W:] = x[b, 0:CHUNK]
            mask[0, :W] = 0.0
        else:
            xs[:] = x[b, start:start + NT]
        in_maps.append({
            "x": np.ascontiguousarray(xs), "w1x": w1xT, "w1z": w1zT,
            "gw8": gw8, "opw": opT, "convw": convw_r, "convb": convb_r,
            "gateb": gateb_r, "cx": cx_r, "cz": cz_r,
            "mask": mask.astype(bf16),
        })
    return in_maps


def kernel(x, norm_w, norm_b, in_proj_w, conv_w, conv_b, gate_w, gate_b,
           out_proj_w, _trace=False, _collect=None):
    x = np.asarray(x, np.float32)
    if "nc" not in _cache:
        _cache["nc"] = _build()
    nc = _cache["nc"]
    in_maps = _prep_host(
        x, np.asarray(norm_w, np.float32), np.asarray(norm_b, np.float32),
        np.asarray(in_proj_w, np.float32), np.asarray(conv_w, np.float32),
        np.asarray(conv_b, np.float32), np.asarray(gate_w, np.float32),
        np.asarray(gate_b, np.float32), np.asarray(out_proj_w, np.float32))
    res = run_bass_kernel_spmd(nc, in_maps, core_ids=list(range(8)), trace=_trace)
    if _collect is not None:
        _collect.append(res)
    out = np.empty((B, L, D), np.float32)
    for core in range(8):
        b, j = core // 4, core % 4
        out[b, j * CHUNK:(j + 1) * CHUNK] = res.results[core]["out"]
    return out


# revision 16
# speedup vs baseline: 1.7808x; 1.7808x over previous
"""GatedLinearRecurrence Trainium2 kernel (8-core SPMD, Bass/Tile).

Strategy: shard (batch=2) x (4 sequence chunks of 1024 tokens) across 8 cores.
Each core processes 1152 tokens: a 128-token warm-up window (recomputed
redundantly; the recurrence decay makes carry-in truncation error ~1e-24)
followed by its 1024 "main" tokens.  No collectives needed.

v2 changes vs baseline:
  * bf16 operands for in_proj / out_proj / transposes (same PE rate as f32r
    at these widths, but half the DMA + SBUF, 1.0 c/row transposes).
  * gate matmul in fp8 e4m3 with MatmulPerfMode.DoubleRow (K=256 per
    instruction, 2x PE throughput).  gw is pre-scaled x32 and xc x4 on the
    fp8 copy; the 1/128 is folded into the sigmoid evacuation scale.
    Simulated end-to-end rel err 0.0049 (gate 2e-2).
  * norm_b folded into a per-output-channel in_proj bias (host-computed
    c = in_proj_w @ norm_b), so transposed x-hat needs no bias and all 8
    d-tiles of a token tile evacuate PSUM in ONE scalar op.
  * z kept in SBUF as silu(z) bf16 (no HBM scratch roundtrip).
  * phase order S2(in_proj-x) -> S3'(gate et interleaved with z et) -> S7
    (out_proj): the PE never waits on the DVE scans, z fills the gaps.
  * scans/bt/yz alternate vector/gpsimd by et parity; conv split across
    vector/gpsimd; out_proj final pass runs kt-inner so the 8 tail
    evacuations stagger instead of serializing.
"""
import sys

for p in ("/opt/trn_rl_repo", "/root/.axon_site/_ro/trn_rl_repo"):
    if p not in sys.path:
        sys.path.insert(0, p)

import numpy as np
import ml_dtypes

import concourse.bass as bass
import concourse.bacc as bacc
import concourse.tile as tile
import concourse.mybir as mybir
from concourse.bass_utils import run_bass_kernel_spmd
from concourse.masks import make_identity

F32 = mybir.dt.float32
BF16 = mybir.dt.bfloat16
FP8 = mybir.dt.float8e4
AF = mybir.ActivationFunctionType
OP = mybir.AluOpType
DR = mybir.MatmulPerfMode.DoubleRow

B, L, D = 2, 4096, 1024
DI = 2048            # d_inner
NT = 1152            # tokens per core (128 warm-up + 1024 main)
W = 128              # warm-up tokens
CHUNK = 1024
NTT = NT // 128      # 9 token tiles
KD = D // 128        # 8 k-tiles over d_model
KC = DI // 128       # 16 k-tiles over d_inner
KC2 = KC // 2        # 8 fp8 k-pair tiles (DoubleRow contracts 256)
TC = 384             # matmul N chunk (3 per core)
NTC = NT // TC
EPS = 1e-5
SW = 32.0            # gate weight fp8 pre-scale
SX = 4.0             # xc fp8 pre-scale

_cache = {}


def _build():
    nc = bacc.Bacc(None, target_bir_lowering=False)

    x_h = nc.dram_tensor("x", [NT, D], F32, kind="ExternalInput")
    w1x_h = nc.dram_tensor("w1x", [D, DI], BF16, kind="ExternalInput")
    w1z_h = nc.dram_tensor("w1z", [D, DI], BF16, kind="ExternalInput")
    gw8_h = nc.dram_tensor("gw8", [KC, 128, KC2 * 2 * 128], FP8, kind="ExternalInput")
    op_h = nc.dram_tensor("opw", [DI, D], BF16, kind="ExternalInput")
    convw_h = nc.dram_tensor("convw", [128, KC * 4], F32, kind="ExternalInput")
    convb_h = nc.dram_tensor("convb", [128, KC], F32, kind="ExternalInput")
    gateb_h = nc.dram_tensor("gateb", [128, KC], F32, kind="ExternalInput")
    cx_h = nc.dram_tensor("cx", [128, KC], F32, kind="ExternalInput")
    cz_h = nc.dram_tensor("cz", [128, KC], F32, kind="ExternalInput")
    mask_h = nc.dram_tensor("mask", [1, W], BF16, kind="ExternalInput")
    out_h = nc.dram_tensor("out", [CHUNK, D], F32, kind="ExternalOutput")

    with tile.TileContext(nc) as tc:
        with tc.tile_pool(name="consts", bufs=1) as consts:

            ident = consts.tile([128, 128], BF16, name="ident")
            make_identity(nc, ident)
            mask_sb = consts.tile([128, W], BF16, name="mask_sb")
            nc.gpsimd.dma_start(
                out=mask_sb,
                in_=bass.AP(tensor=mask_h, offset=0, ap=[[0, 128], [1, W]]))
            convw = consts.tile([128, KC * 4], F32, name="convw")
            nc.gpsimd.dma_start(out=convw, in_=convw_h.ap())
            convb = consts.tile([128, KC], F32, name="convb")
            nc.gpsimd.dma_start(out=convb, in_=convb_h.ap())
            gateb = consts.tile([128, KC], F32, name="gateb")
            nc.gpsimd.dma_start(out=gateb, in_=gateb_h.ap())
            cxb = consts.tile([128, KC], F32, name="cxb")
            nc.gpsimd.dma_start(out=cxb, in_=cx_h.ap())
            czb = consts.tile([128, KC], F32, name="czb")
            nc.gpsimd.dma_start(out=czb, in_=cz_h.ap())
            eps_t = consts.tile([128, 1], F32, name="eps_t")
            nc.vector.memset(eps_t, EPS)

            # long-lived activation stores
            with tc.tile_pool(name="xcp", bufs=1) as xcp, \
                 tc.tile_pool(name="xc8p", bufs=1) as xc8p, \
                 tc.tile_pool(name="gws", bufs=4) as gs, \
                 tc.tile_pool(name="xT", bufs=1) as xTp:

                xc = [xcp.tile([128, NT], BF16, name=f"xct{e}") for e in range(KC)]
                xc8 = [xc8p.tile([128, 2, NT], FP8, name=f"xc8t{j}")
                       for j in range(KC2)]
                # x-hat-T per chunk: [128 d-part, KD d-tiles, TC tokens]
                xT = [xTp.tile([128, KD, TC], BF16, name=f"xTt{c_}")
                      for c_ in range(NTC)]

                # prefetch first gate-weight tiles early (gpsimd queue head)
                gts = {}
                for et in range(2):
                    gt = gs.tile([128, KC2, 2, 128], FP8, tag="gw", name=f"gt{et}")
                    nc.gpsimd.dma_start(out=gt, in_=gw8_h.ap()[et])
                    gts[et] = gt

                # ---- S1-S2: LN, transpose, in_proj x-half, conv, silu ----
                with tc.tile_pool(name="s1roll", bufs=2) as s1r, \
                     tc.tile_pool(name="stat", bufs=4) as stp, \
                     tc.tile_pool(name="w1s", bufs=3) as ws, \
                     tc.tile_pool(name="psmm", bufs=4, space="PSUM") as psmm, \
                     tc.tile_pool(name="pstr", bufs=3, space="PSUM") as pstr:

                    for it in range(NTT):
                        tc3, col = it // 3, (it % 3) * 128
                        xt = s1r.tile([128, D], F32, tag="xt", bufs=4, name="xt")
                        eng = nc.sync if it % 2 == 0 else nc.scalar
                        eng.dma_start(out=xt, in_=x_h.ap()[it * 128:(it + 1) * 128, :])
                        stats = stp.tile([128, 2, 6], F32, tag="stats", name="stats")
                        nc.vector.bn_stats(out=stats[:, 0, :], in_=xt[:, 0:512])
                        nc.vector.bn_stats(out=stats[:, 1, :], in_=xt[:, 512:1024])
                        mv = stp.tile([128, 2], F32, tag="mv", name="mv")
                        nc.vector.bn_aggr(out=mv, in_=stats)
                        rstd = stp.tile([128, 1], F32, tag="rstd", name="rstd")
                        nc.scalar.activation(out=rstd, in_=mv[:, 1:2], func=AF.Sqrt,
                                             bias=eps_t, scale=1.0)
                        nc.vector.reciprocal(out=rstd, in_=rstd)
                        nmr = stp.tile([128, 1], F32, tag="nmr", name="nmr")
                        nc.vector.tensor_scalar(out=nmr, in0=mv[:, 0:1],
                                                scalar1=rstd, scalar2=-1.0,
                                                op0=OP.mult, op1=OP.mult)
                        xh = s1r.tile([128, D], BF16, tag="xh", bufs=3, name="xh")
                        # LN apply on the Scalar engine: x*rstd - mu*rstd
                        nc.scalar.activation(out=xh, in_=xt, func=AF.Identity,
                                             scale=rstd, bias=nmr)
                        pst = pstr.tile([128, KD, 128], BF16, tag="tr", name="pst")
                        # one accumulation group over the 8 disjoint d-tile
                        # regions: first write after start zeroes per-byte
                        for d_ in range(KD):
                            nc.tensor.matmul(pst[:, d_, :],
                                             xh[:, d_ * 128:(d_ + 1) * 128], ident,
                                             start=(d_ == 0), stop=(d_ == KD - 1),
                                             is_transpose=True,
                                             skip_group_check=True)
                        # one evacuation for all 8 d-tiles of this token tile
                        nc.scalar.activation(
                            out=xT[tc3][:, :, col:col + 128], in_=pst,
                            func=AF.Identity, scale=1.0)

                    # in_proj x-half + conv + silu + warm-up mask + fp8 cast.
                    NW = 4
                    order = [(e, c) for c in range(NTC) for e in range(NW)]
                    order += [(e, c) for e in range(NW, KC) for c in range(NTC)]
                    wts, xins = {}, {}

                    def s2_chain(et, tc3):
                        if tc3 == 0:
                            wt = ws.tile([128, KD, 128], BF16, tag="w1",
                                         bufs=6, name=f"wt{et}")
                            nc.sync.dma_start(
                                out=wt,
                                in_=w1x_h.ap()[:, et * 128:(et + 1) * 128]
                                .rearrange("(kt p) e -> p kt e", p=128))
                            wts[et] = wt
                            xin = s1r.tile([128, NT + 3], BF16, tag="xin",
                                           bufs=NW + 2, name=f"xin{et}")
                            nc.vector.memset(xin[:, 0:3], 0.0)
                            xins[et] = xin
                        ps = psmm.tile([128, TC], F32, tag="mm", name="ps")
                        for kt in range(KD):
                            nc.tensor.matmul(
                                ps, wts[et][:, kt, :], xT[tc3][:, kt, :],
                                start=(kt == 0), stop=(kt == KD - 1))
                        nc.scalar.activation(
                            out=xins[et][:, 3 + tc3 * TC: 3 + (tc3 + 1) * TC],
                            in_=ps, func=AF.Identity, bias=cxb[:, et:et + 1],
                            scale=1.0)
                        if tc3 == NTC - 1:
                            xin = xins.pop(et)
                            tmp = s1r.tile([128, NT], BF16, tag="ctmp", bufs=3,
                                           name="ctmp")
                            nc.vector.tensor_scalar_mul(
                                tmp, xin[:, 0:NT], convw[:, et * 4:et * 4 + 1])
                            for k in range(1, 4):
                                nc.vector.scalar_tensor_tensor(
                                    out=tmp, in0=xin[:, k:k + NT],
                                    scalar=convw[:, et * 4 + k:et * 4 + k + 1],
                                    in1=tmp, op0=OP.mult, op1=OP.add)
                            nc.scalar.activation(out=xc[et], in_=tmp, func=AF.Silu,
                                                 bias=convb[:, et:et + 1], scale=1.0)
                            # mask is non-unit only on the warm-up columns
                            nc.vector.tensor_mul(
                                xc[et][:, 0:W], xc[et][:, 0:W], mask_sb)
                            # fp8 copy (x SX) for the gate matmul rhs
                            nc.scalar.activation(
                                out=xc8[et // 2][:, et % 2, :], in_=xc[et],
                                func=AF.Copy, scale=SX)

                    for et, tc3 in order:
                        s2_chain(et, tc3)

                # ---- S3': per et: gate (fp8 DoubleRow) + z-half (bf16),
                # sigmoid/scan/yz trail on scalar + vector/gpsimd. ----
                with tc.tile_pool(name="yp", bufs=1) as yp:
                    ych = [None] * KC
                    with tc.tile_pool(name="w1zs", bufs=3) as wzs, \
                         tc.tile_pool(name="zsil", bufs=4) as zsp, \
                         tc.tile_pool(name="ach", bufs=3) as ayp, \
                         tc.tile_pool(name="btr", bufs=2) as btp, \
                         tc.tile_pool(name="sgr", bufs=4) as sgp, \
                         tc.tile_pool(name="psg", bufs=2, space="PSUM") as psg, \
                         tc.tile_pool(name="psz", bufs=3, space="PSUM") as psz:

                        zs = {}

                        def z_half(et):
                            wt = wzs.tile([128, KD, 128], BF16, tag="wz", name="wtz")
                            nc.sync.dma_start(
                                out=wt,
                                in_=w1z_h.ap()[:, et * 128:(et + 1) * 128]
                                .rearrange("(kt p) e -> p kt e", p=128))
                            zt = zsp.tile([128, CHUNK], BF16, tag="zs", name="zst")
                            zs[et] = zt
                            for tc3 in range(NTC):
                                lo = W if tc3 == 0 else tc3 * TC   # NT-space
                                n = (tc3 + 1) * TC - lo
                                ps = psz.tile([128, TC], F32, tag="zmm", name="pszt")
                                for kt in range(KD):
                                    nc.tensor.matmul(
                                        ps[:, 0:n], wt[:, kt, :],
                                        xT[tc3][:, kt, TC - n:TC],
                                        start=(kt == 0), stop=(kt == KD - 1))
                                # silu(v) = v*sigmoid(v): scalar stays on the
                                # Sigmoid table all phase, vector fuses the
                                # add+mult straight from PSUM
                                sg = sgp.tile([128, TC], BF16, tag="sg", name="sg")
                                nc.scalar.activation(
                                    out=sg[:, 0:n], in_=ps[:, 0:n], func=AF.Sigmoid,
                                    bias=czb[:, et:et + 1], scale=1.0)
                                nc.vector.scalar_tensor_tensor(
                                    out=zt[:, lo - W:(tc3 + 1) * TC - W],
                                    in0=ps[:, 0:n], scalar=czb[:, et:et + 1],
                                    in1=sg[:, 0:n], op0=OP.add, op1=OP.mult)

                        def gate(et):
                            gt = gts.pop(et)
                            if et + 2 < KC:
                                g2 = gs.tile([128, KC2, 2, 128], FP8, tag="gw",
                                             name=f"gt{et + 2}")
                                nc.gpsimd.dma_start(out=g2, in_=gw8_h.ap()[et + 2])
                                gts[et + 2] = g2
                            a_t = ayp.tile([128, NT], BF16, tag="ach", name="ach")
                            for tc3 in range(NTC):
                                ps = psg.tile([128, TC], F32, tag="gmm", name="psgt")
                                for j in range(KC2):
                                    nc.tensor.matmul(
                                        ps, gt[:, j], xc8[j][:, :, tc3 * TC:(tc3 + 1) * TC],
                                        start=(j == 0), stop=(j == KC2 - 1),
                                        perf_mode=DR)
                                nc.scalar.activation(
                                    out=a_t[:, tc3 * TC:(tc3 + 1) * TC], in_=ps,
                                    func=AF.Sigmoid,
                                    bias=gateb[:, et:et + 1], scale=1.0 / (SW * SX))
                            # full-width bt/scan/yz: fewer DVE ops, no chaining
                            bt = btp.tile([128, NT], BF16, tag="bt", name="bt")
                            nc.vector.scalar_tensor_tensor(
                                out=bt, in0=a_t, scalar=1.0, in1=xc[et],
                                op0=OP.subtract, op1=OP.mult)
                            y_t = yp.tile([128, NT], BF16, name=f"y{et}")
                            nc.vector.tensor_tensor_scan(
                                out=y_t, data0=a_t, data1=bt, initial=0.0,
                                op0=OP.mult, op1=OP.add)
                            zt = zs.pop(et)
                            nc.vector.tensor_mul(y_t[:, W:NT], y_t[:, W:NT], zt)
                            ych[et] = y_t

                        z_half(0)
                        z_half(1)
                        for et in range(KC):
                            gate(et)
                            if et + 2 < KC:
                                z_half(et + 2)

                    # ---- S7: out_proj + residual. ----
                    NTB = CHUNK // 128

                    def yslice(kt, tb):
                        col = W + tb * 128          # absolute column in [0, NT)
                        return ych[kt][:, col:col + 128]

                    with tc.tile_pool(name="ops", bufs=18) as opp, \
                         tc.tile_pool(name="s7roll", bufs=6) as s7r, \
                         tc.tile_pool(name="s7res", bufs=8) as s7x, \
                         tc.tile_pool(name="psop", bufs=8, space="PSUM") as psop:
                        for nb in range(2):
                            xres = {}
                            for tb in range(NTB):
                                xres[tb] = s7x.tile([128, 512], F32, tag="xres",
                                                    name=f"xres{tb}")
                                nc.scalar.dma_start(
                                    out=xres[tb],
                                    in_=x_h.ap()[W + tb * 128:W + (tb + 1) * 128,
                                                 nb * 512:(nb + 1) * 512])
                            pss = [psop.tile([128, 512], F32, tag="op",
                                             name=f"pso{tb}") for tb in range(NTB)]
                            opts = []
                            for kt in range(KC):
                                opt = opp.tile([128, 512], BF16, tag="opw",
                                               name="opt")
                                nc.gpsimd.dma_start(
                                    out=opt,
                                    in_=op_h.ap()[kt * 128:(kt + 1) * 128,
                                                  nb * 512:(nb + 1) * 512])
                                opts.append(opt)
                            if nb == 0:
                                # kt-outer: max stationary reuse
                                for kt in range(KC):
                                    for tb in range(NTB):
                                        nc.tensor.matmul(
                                            pss[tb], yslice(kt, tb), opts[kt],
                                            start=(kt == 0), stop=(kt == KC - 1))
                            else:
                                # kt-inner: stagger the final evacuations
                                for tb in range(NTB):
                                    for kt in range(KC):
                                        nc.tensor.matmul(
                                            pss[tb], yslice(kt, tb), opts[kt],
                                            start=(kt == 0), stop=(kt == KC - 1))
                            for tb in range(NTB):
                                oh = s7r.tile([128, 512], F32, tag="oh", name="oh")
                                nc.vector.tensor_sub(oh, xres[tb], pss[tb])
                                deng = nc.sync if tb % 2 == 0 else nc.scalar
                                deng.dma_start(
                                    out=out_h.ap()[tb * 128:(tb + 1) * 128,
                                                   nb * 512:(nb + 1) * 512],
                                    in_=oh)

    nc.compile()
    return nc


def _prep_host(x, norm_w, norm_b, in_proj_w, conv_w, conv_b, gate_w, gate_b,
               out_proj_w):
    bf16 = ml_dtypes.bfloat16
    fp8 = ml_dtypes.float8_e4m3
    w1f = (in_proj_w * norm_w[None, :]).astype(np.float32)
    cbias = (in_proj_w.astype(np.float64) @ norm_b.astype(np.float64)).astype(np.float32)
    w1xT = np.ascontiguousarray(w1f[:DI].T).astype(bf16)      # [D, DI]
    w1zT = np.ascontiguousarray(w1f[DI:].T).astype(bf16)      # [D, DI]
    # gw8[et, p, j, s, m] = gw[et*128+m, j*256+s*128+p] * SW
    gwT = np.ascontiguousarray(gate_w.T * SW)                 # [c_in, e_out]
    gw8 = gwT.reshape(KC2, 2, 128, KC, 128).transpose(3, 2, 0, 1, 4)
    gw8 = np.ascontiguousarray(gw8.reshape(KC, 128, KC2 * 2 * 128)).astype(fp8)
    opT = np.ascontiguousarray(out_proj_w.T).astype(bf16)     # [DI, D]
    convw_r = np.ascontiguousarray(
        conv_w.reshape(KC, 128, 4).transpose(1, 0, 2).reshape(128, KC * 4))
    convb_r = np.ascontiguousarray(conv_b.reshape(KC, 128).T)
    gateb_r = np.ascontiguousarray(gate_b.reshape(KC, 128).T)
    cx_r = np.ascontiguousarray(cbias[:DI].reshape(KC, 128).T)
    cz_r = np.ascontiguousarray(cbias[DI:].reshape(KC, 128).T)

    in_maps = []
    for core in range(8):
        b, j = core // 4, core % 4
        xs = np.zeros((NT, D), np.float32)
        start = j * CHUNK - W
        mask = np.ones((1, W), np.float32)
        if j == 0:
            xs[W:] = x[b, 0:CHUNK]
            mask[0, :W] = 0.0
        else:
            xs[:] = x[b, start:start + NT]
        in_maps.append({
            "x": np.ascontiguousarray(xs), "w1x": w1xT, "w1z": w1zT,
            "gw8": gw8, "opw": opT, "convw": convw_r, "convb": convb_r,
            "gateb": gateb_r, "cx": cx_r, "cz": cz_r,
            "mask": mask.astype(bf16),
        })
    return in_maps


def kernel(x, norm_w, norm_b, in_proj_w, conv_w, conv_b, gate_w, gate_b,
           out_proj_w, _trace=False, _collect=None):
    x = np.asarray(x, np.float32)
    if "nc" not in _cache:
        _cache["nc"] = _build()
    nc = _cache["nc"]
    in_maps = _prep_host(
        x, np.asarray(norm_w, np.float32), np.asarray(norm_b, np.float32),
        np.asarray(in_proj_w, np.float32), np.asarray(conv_w, np.float32),
        np.asarray(conv_b, np.float32), np.asarray(gate_w, np.float32),
        np.asarray(gate_b, np.float32), np.asarray(out_proj_w, np.float32))
    res = run_bass_kernel_spmd(nc, in_maps, core_ids=list(range(8)), trace=_trace)
    if _collect is not None:
        _collect.append(res)
    out = np.empty((B, L, D), np.float32)
    for core in range(8):
        b, j = core // 4, core % 4
        out[b, j * CHUNK:(j + 1) * CHUNK] = res.results[core]["out"]
    return out


# revision 18
# speedup vs baseline: 1.8924x; 1.0627x over previous
"""GatedLinearRecurrence Trainium2 kernel (8-core SPMD, Bass/Tile).

Strategy: shard (batch=2) x (4 sequence chunks of 1024 tokens) across 8 cores.
Each core processes 1152 tokens: a 128-token warm-up window (recomputed
redundantly; the recurrence decay makes carry-in truncation error ~1e-24)
followed by its 1024 "main" tokens.  No collectives needed.

v2 changes vs baseline:
  * bf16 operands for in_proj / out_proj / transposes (same PE rate as f32r
    at these widths, but half the DMA + SBUF, 1.0 c/row transposes).
  * gate matmul in fp8 e4m3 with MatmulPerfMode.DoubleRow (K=256 per
    instruction, 2x PE throughput).  gw is pre-scaled x32 and xc x4 on the
    fp8 copy; the 1/128 is folded into the sigmoid evacuation scale.
    Simulated end-to-end rel err 0.0049 (gate 2e-2).
  * norm_b folded into a per-output-channel in_proj bias (host-computed
    c = in_proj_w @ norm_b), so transposed x-hat needs no bias and all 8
    d-tiles of a token tile evacuate PSUM in ONE scalar op.
  * z kept in SBUF as silu(z) bf16 (no HBM scratch roundtrip).
  * phase order S2(in_proj-x) -> S3'(gate et interleaved with z et) -> S7
    (out_proj): the PE never waits on the DVE scans, z fills the gaps.
  * scans/bt/yz alternate vector/gpsimd by et parity; conv split across
    vector/gpsimd; out_proj final pass runs kt-inner so the 8 tail
    evacuations stagger instead of serializing.
"""
import sys

for p in ("/opt/trn_rl_repo", "/root/.axon_site/_ro/trn_rl_repo"):
    if p not in sys.path:
        sys.path.insert(0, p)

import numpy as np
import ml_dtypes

import concourse.bass as bass
import concourse.bacc as bacc
import concourse.tile as tile
import concourse.mybir as mybir
from concourse.bass_utils import run_bass_kernel_spmd
from concourse.masks import make_identity

F32 = mybir.dt.float32
BF16 = mybir.dt.bfloat16
FP8 = mybir.dt.float8e4
AF = mybir.ActivationFunctionType
OP = mybir.AluOpType
DR = mybir.MatmulPerfMode.DoubleRow

B, L, D = 2, 4096, 1024
DI = 2048            # d_inner
NT = 1152            # tokens per core (128 warm-up + 1024 main)
W = 128              # warm-up tokens
CHUNK = 1024
NTT = NT // 128      # 9 token tiles
KD = D // 128        # 8 k-tiles over d_model
KC = DI // 128       # 16 k-tiles over d_inner
KC2 = KC // 2        # 8 fp8 k-pair tiles (DoubleRow contracts 256)
TC = 384             # matmul N chunk (3 per core)
NTC = NT // TC
EPS = 1e-5
SW = 32.0            # gate weight fp8 pre-scale
SX = 4.0             # xc fp8 pre-scale

_cache = {}


def _build():
    nc = bacc.Bacc(None, target_bir_lowering=False)

    x_h = nc.dram_tensor("x", [NT, D], F32, kind="ExternalInput")
    w1x_h = nc.dram_tensor("w1x", [KC, 128, KD * 128], BF16, kind="ExternalInput")
    w1z_h = nc.dram_tensor("w1z", [KC, 128, KD * 128], BF16, kind="ExternalInput")
    gw8_h = nc.dram_tensor("gw8", [KC, 128, KC2 * 2 * 128], FP8, kind="ExternalInput")
    op_h = nc.dram_tensor("opw", [2, KC, 128, 512], BF16, kind="ExternalInput")
    convw_h = nc.dram_tensor("convw", [128, KC * 4], F32, kind="ExternalInput")
    convb_h = nc.dram_tensor("convb", [128, KC], F32, kind="ExternalInput")
    gateb_h = nc.dram_tensor("gateb", [128, KC], F32, kind="ExternalInput")
    cx_h = nc.dram_tensor("cx", [128, KC], F32, kind="ExternalInput")
    cz_h = nc.dram_tensor("cz", [128, KC], F32, kind="ExternalInput")
    mask_h = nc.dram_tensor("mask", [1, W], BF16, kind="ExternalInput")
    out_h = nc.dram_tensor("out", [CHUNK, D], F32, kind="ExternalOutput")

    with tile.TileContext(nc) as tc:
        with tc.tile_pool(name="consts", bufs=1) as consts:

            ident = consts.tile([128, 128], BF16, name="ident")
            make_identity(nc, ident)
            mask_sb = consts.tile([128, W], BF16, name="mask_sb")
            nc.gpsimd.dma_start(
                out=mask_sb,
                in_=bass.AP(tensor=mask_h, offset=0, ap=[[0, 128], [1, W]]))
            convw = consts.tile([128, KC * 4], F32, name="convw")
            nc.gpsimd.dma_start(out=convw, in_=convw_h.ap())
            convb = consts.tile([128, KC], F32, name="convb")
            nc.gpsimd.dma_start(out=convb, in_=convb_h.ap())
            gateb = consts.tile([128, KC], F32, name="gateb")
            nc.gpsimd.dma_start(out=gateb, in_=gateb_h.ap())
            cxb = consts.tile([128, KC], F32, name="cxb")
            nc.gpsimd.dma_start(out=cxb, in_=cx_h.ap())
            czb = consts.tile([128, KC], F32, name="czb")
            nc.gpsimd.dma_start(out=czb, in_=cz_h.ap())
            eps_t = consts.tile([128, 1], F32, name="eps_t")
            nc.vector.memset(eps_t, EPS)

            # long-lived activation stores
            with tc.tile_pool(name="xcp", bufs=1) as xcp, \
                 tc.tile_pool(name="xc8p", bufs=1) as xc8p, \
                 tc.tile_pool(name="gws", bufs=4) as gs, \
                 tc.tile_pool(name="xT", bufs=1) as xTp:

                xc = [xcp.tile([128, NT], BF16, name=f"xct{e}") for e in range(KC)]
                xc8 = [xc8p.tile([128, 2, NT], FP8, name=f"xc8t{j}")
                       for j in range(KC2)]
                # x-hat-T per chunk: [128 d-part, KD d-tiles, TC tokens]
                xT = [xTp.tile([128, KD, TC], BF16, name=f"xTt{c_}")
                      for c_ in range(NTC)]

                # prefetch first gate-weight tiles early (gpsimd queue head)
                gts = {}
                for et in range(2):
                    gt = gs.tile([128, KC2, 2, 128], FP8, tag="gw", name=f"gt{et}")
                    nc.gpsimd.dma_start(out=gt, in_=gw8_h.ap()[et])
                    gts[et] = gt

                # ---- S1-S2: LN, transpose, in_proj x-half, conv, silu ----
                with tc.tile_pool(name="s1roll", bufs=2) as s1r, \
                     tc.tile_pool(name="stat", bufs=4) as stp, \
                     tc.tile_pool(name="w1s", bufs=3) as ws, \
                     tc.tile_pool(name="psmm", bufs=4, space="PSUM") as psmm, \
                     tc.tile_pool(name="pstr", bufs=3, space="PSUM") as pstr:

                    for it in range(NTT):
                        tc3, col = it // 3, (it % 3) * 128
                        xt = s1r.tile([128, D], F32, tag="xt", bufs=4, name="xt")
                        qs = (nc.sync, nc.scalar, nc.gpsimd)
                        qs[it % 3].dma_start(
                            out=xt[:, 0:512],
                            in_=x_h.ap()[it * 128:(it + 1) * 128, 0:512])
                        qs[(it + 1) % 3].dma_start(
                            out=xt[:, 512:1024],
                            in_=x_h.ap()[it * 128:(it + 1) * 128, 512:1024])
                        stats = stp.tile([128, 2, 6], F32, tag="stats", name="stats")
                        nc.vector.bn_stats(out=stats[:, 0, :], in_=xt[:, 0:512])
                        nc.vector.bn_stats(out=stats[:, 1, :], in_=xt[:, 512:1024])
                        mv = stp.tile([128, 2], F32, tag="mv", name="mv")
                        nc.vector.bn_aggr(out=mv, in_=stats)
                        rstd = stp.tile([128, 1], F32, tag="rstd", name="rstd")
                        nc.scalar.activation(out=rstd, in_=mv[:, 1:2], func=AF.Sqrt,
                                             bias=eps_t, scale=1.0)
                        nc.vector.reciprocal(out=rstd, in_=rstd)
                        nmr = stp.tile([128, 1], F32, tag="nmr", name="nmr")
                        nc.vector.tensor_scalar(out=nmr, in0=mv[:, 0:1],
                                                scalar1=rstd, scalar2=-1.0,
                                                op0=OP.mult, op1=OP.mult)
                        xh = s1r.tile([128, D], BF16, tag="xh", bufs=3, name="xh")
                        # LN apply on the Scalar engine: x*rstd - mu*rstd
                        nc.scalar.activation(out=xh, in_=xt, func=AF.Identity,
                                             scale=rstd, bias=nmr)
                        pst = pstr.tile([128, KD, 128], BF16, tag="tr", name="pst")
                        # one accumulation group over the 8 disjoint d-tile
                        # regions: first write after start zeroes per-byte
                        for d_ in range(KD):
                            nc.tensor.matmul(pst[:, d_, :],
                                             xh[:, d_ * 128:(d_ + 1) * 128], ident,
                                             start=(d_ == 0), stop=(d_ == KD - 1),
                                             is_transpose=True,
                                             skip_group_check=True)
                        # one evacuation for all 8 d-tiles of this token tile
                        nc.scalar.activation(
                            out=xT[tc3][:, :, col:col + 128], in_=pst,
                            func=AF.Identity, scale=1.0)

                    # in_proj x-half + conv + silu + warm-up mask + fp8 cast.
                    NW = 6
                    order = [(e, c) for c in range(NTC) for e in range(NW)]
                    order += [(e, c) for e in range(NW, KC) for c in range(NTC)]
                    wts, xins = {}, {}

                    def s2_chain(et, tc3):
                        if tc3 == 0:
                            wt = ws.tile([128, KD, 128], BF16, tag="w1",
                                         bufs=6, name=f"wt{et}")
                            nc.sync.dma_start(out=wt, in_=w1x_h.ap()[et])
                            wts[et] = wt
                            xin = s1r.tile([128, NT + 3], BF16, tag="xin",
                                           bufs=NW + 2, name=f"xin{et}")
                            nc.vector.memset(xin[:, 0:3], 0.0)
                            xins[et] = xin
                        ps = psmm.tile([128, TC], F32, tag="mm", name="ps")
                        for kt in range(KD):
                            nc.tensor.matmul(
                                ps, wts[et][:, kt, :], xT[tc3][:, kt, :],
                                start=(kt == 0), stop=(kt == KD - 1))
                        nc.scalar.activation(
                            out=xins[et][:, 3 + tc3 * TC: 3 + (tc3 + 1) * TC],
                            in_=ps, func=AF.Identity, bias=cxb[:, et:et + 1],
                            scale=1.0)
                        if tc3 == NTC - 1:
                            xin = xins.pop(et)
                            tmp = s1r.tile([128, NT], BF16, tag="ctmp", bufs=3,
                                           name="ctmp")
                            nc.vector.tensor_scalar_mul(
                                tmp, xin[:, 0:NT], convw[:, et * 4:et * 4 + 1])
                            for k in range(1, 4):
                                nc.vector.scalar_tensor_tensor(
                                    out=tmp, in0=xin[:, k:k + NT],
                                    scalar=convw[:, et * 4 + k:et * 4 + k + 1],
                                    in1=tmp, op0=OP.mult, op1=OP.add)
                            nc.scalar.activation(out=xc[et], in_=tmp, func=AF.Silu,
                                                 bias=convb[:, et:et + 1], scale=1.0)
                            # mask is non-unit only on the warm-up columns
                            nc.vector.tensor_mul(
                                xc[et][:, 0:W], xc[et][:, 0:W], mask_sb)
                            # fp8 copy (x SX) for the gate matmul rhs
                            nc.scalar.activation(
                                out=xc8[et // 2][:, et % 2, :], in_=xc[et],
                                func=AF.Copy, scale=SX)

                    for et, tc3 in order:
                        s2_chain(et, tc3)

                # ---- S3': per et: gate (fp8 DoubleRow) + z-half (bf16),
                # sigmoid/scan/yz trail on scalar + vector/gpsimd. ----
                with tc.tile_pool(name="yp", bufs=1) as yp, \
                     tc.tile_pool(name="ops", bufs=18) as opp, \
                     tc.tile_pool(name="s7res", bufs=16) as s7x:
                    ych = [None] * KC
                    NTB = CHUNK // 128
                    opts0, xres0 = [], []
                    with tc.tile_pool(name="w1zs", bufs=3) as wzs, \
                         tc.tile_pool(name="zsil", bufs=4) as zsp, \
                         tc.tile_pool(name="ach", bufs=3) as ayp, \
                         tc.tile_pool(name="btr", bufs=2) as btp, \
                         tc.tile_pool(name="sgr", bufs=4) as sgp, \
                         tc.tile_pool(name="psg", bufs=2, space="PSUM") as psg, \
                         tc.tile_pool(name="psz", bufs=3, space="PSUM") as psz:

                        zs = {}

                        def z_half(et):
                            wt = wzs.tile([128, KD, 128], BF16, tag="wz", name="wtz")
                            nc.sync.dma_start(out=wt, in_=w1z_h.ap()[et])
                            zt = zsp.tile([128, CHUNK], BF16, tag="zs", name="zst")
                            zs[et] = zt
                            for tc3 in range(NTC):
                                lo = W if tc3 == 0 else tc3 * TC   # NT-space
                                n = (tc3 + 1) * TC - lo
                                ps = psz.tile([128, TC], F32, tag="zmm", name="pszt")
                                for kt in range(KD):
                                    nc.tensor.matmul(
                                        ps[:, 0:n], wt[:, kt, :],
                                        xT[tc3][:, kt, TC - n:TC],
                                        start=(kt == 0), stop=(kt == KD - 1))
                                # silu(v) = v*sigmoid(v): scalar stays on the
                                # Sigmoid table all phase, vector fuses the
                                # add+mult straight from PSUM
                                sg = sgp.tile([128, TC], BF16, tag="sg", name="sg")
                                nc.scalar.activation(
                                    out=sg[:, 0:n], in_=ps[:, 0:n], func=AF.Sigmoid,
                                    bias=czb[:, et:et + 1], scale=1.0)
                                nc.vector.scalar_tensor_tensor(
                                    out=zt[:, lo - W:(tc3 + 1) * TC - W],
                                    in0=ps[:, 0:n], scalar=czb[:, et:et + 1],
                                    in1=sg[:, 0:n], op0=OP.add, op1=OP.mult)

                        def gate(et):
                            gt = gts.pop(et)
                            if et + 2 < KC:
                                g2 = gs.tile([128, KC2, 2, 128], FP8, tag="gw",
                                             name=f"gt{et + 2}")
                                nc.gpsimd.dma_start(out=g2, in_=gw8_h.ap()[et + 2])
                                gts[et + 2] = g2
                            # prefetch out_proj nb=0 weights + residual rows
                            opt = opp.tile([128, 512], BF16, tag="opw", name="opt")
                            nc.gpsimd.dma_start(out=opt, in_=op_h.ap()[0, et])
                            opts0.append(opt)
                            if et >= 8:
                                tb = et - 8
                                xr = s7x.tile([128, 512], F32, tag="xres",
                                              name=f"xres{tb}")
                                nc.scalar.dma_start(
                                    out=xr,
                                    in_=x_h.ap()[W + tb * 128:W + (tb + 1) * 128,
                                                 0:512])
                                xres0.append(xr)
                            a_t = ayp.tile([128, NT], BF16, tag="ach", name="ach")
                            for tc3 in range(NTC):
                                ps = psg.tile([128, TC], F32, tag="gmm", name="psgt")
                                for j in range(KC2):
                                    nc.tensor.matmul(
                                        ps, gt[:, j], xc8[j][:, :, tc3 * TC:(tc3 + 1) * TC],
                                        start=(j == 0), stop=(j == KC2 - 1),
                                        perf_mode=DR)
                                nc.scalar.activation(
                                    out=a_t[:, tc3 * TC:(tc3 + 1) * TC], in_=ps,
                                    func=AF.Sigmoid,
                                    bias=gateb[:, et:et + 1], scale=1.0 / (SW * SX))
                            # full-width bt/scan/yz: fewer DVE ops, no chaining
                            bt = btp.tile([128, NT], BF16, tag="bt", name="bt")
                            nc.vector.scalar_tensor_tensor(
                                out=bt, in0=a_t, scalar=1.0, in1=xc[et],
                                op0=OP.subtract, op1=OP.mult)
                            y_t = yp.tile([128, NT], BF16, name=f"y{et}")
                            nc.vector.tensor_tensor_scan(
                                out=y_t, data0=a_t, data1=bt, initial=0.0,
                                op0=OP.mult, op1=OP.add)
                            zt = zs.pop(et)
                            nc.vector.tensor_mul(y_t[:, W:NT], y_t[:, W:NT], zt)
                            ych[et] = y_t

                        z_half(0)
                        z_half(1)
                        for et in range(KC):
                            gate(et)
                            if et + 2 < KC:
                                z_half(et + 2)

                    # ---- S7: out_proj + residual. ----

                    def yslice(kt, tb):
                        col = W + tb * 128          # absolute column in [0, NT)
                        return ych[kt][:, col:col + 128]

                    with tc.tile_pool(name="s7roll", bufs=6) as s7r, \
                         tc.tile_pool(name="psop", bufs=8, space="PSUM") as psop:
                        for nb in range(2):
                            if nb == 0:
                                xres = xres0
                                opts = opts0
                            else:
                                xres = []
                                for tb in range(NTB):
                                    xr = s7x.tile([128, 512], F32, tag="xres",
                                                  name=f"xres{tb}")
                                    nc.scalar.dma_start(
                                        out=xr,
                                        in_=x_h.ap()[W + tb * 128:W + (tb + 1) * 128,
                                                     512:1024])
                                    xres.append(xr)
                                opts = []
                                for kt in range(KC):
                                    opt = opp.tile([128, 512], BF16, tag="opw",
                                                   name="opt")
                                    nc.gpsimd.dma_start(out=opt,
                                                        in_=op_h.ap()[1, kt])
                                    opts.append(opt)
                            pss = [psop.tile([128, 512], F32, tag="op",
                                             name=f"pso{tb}") for tb in range(NTB)]
                            if nb == 0:
                                # kt-outer: max stationary reuse
                                for kt in range(KC):
                                    for tb in range(NTB):
                                        nc.tensor.matmul(
                                            pss[tb], yslice(kt, tb), opts[kt],
                                            start=(kt == 0), stop=(kt == KC - 1))
                            else:
                                # kt-inner: stagger the final evacuations
                                for tb in range(NTB):
                                    for kt in range(KC):
                                        nc.tensor.matmul(
                                            pss[tb], yslice(kt, tb), opts[kt],
                                            start=(kt == 0), stop=(kt == KC - 1))
                            for tb in range(NTB):
                                oh = s7r.tile([128, 512], F32, tag="oh", name="oh")
                                nc.vector.tensor_sub(oh, xres[tb], pss[tb])
                                deng = nc.sync if tb % 2 == 0 else nc.scalar
                                deng.dma_start(
                                    out=out_h.ap()[tb * 128:(tb + 1) * 128,
                                                   nb * 512:(nb + 1) * 512],
                                    in_=oh)

    nc.compile()
    return nc


def _prep_host(x, norm_w, norm_b, in_proj_w, conv_w, conv_b, gate_w, gate_b,
               out_proj_w):
    bf16 = ml_dtypes.bfloat16
    fp8 = ml_dtypes.float8_e4m3
    w1f = (in_proj_w * norm_w[None, :]).astype(np.float32)
    cbias = (in_proj_w.astype(np.float64) @ norm_b.astype(np.float64)).astype(np.float32)
    # w1 tiles pre-arranged [et, p, kt*128+e]: w1r[et, p, kt, e] = w1f[e_out=et*128+e, d=kt*128+p]
    def w1_tiles(wh):                                          # wh [DI, D]
        t = wh.reshape(KC, 128, KD, 128).transpose(0, 3, 2, 1)  # et, p, kt, e
        return np.ascontiguousarray(t.reshape(KC, 128, KD * 128)).astype(bf16)
    w1xT = w1_tiles(w1f[:DI])
    w1zT = w1_tiles(w1f[DI:])
    # gw8[et, p, j, s, m] = gw[et*128+m, j*256+s*128+p] * SW
    gwT = np.ascontiguousarray(gate_w.T * SW)                 # [c_in, e_out]
    gw8 = gwT.reshape(KC2, 2, 128, KC, 128).transpose(3, 2, 0, 1, 4)
    gw8 = np.ascontiguousarray(gw8.reshape(KC, 128, KC2 * 2 * 128)).astype(fp8)
    # opw tiles [nb, kt, p, e] = out_proj_w[e_out=nb*512+e, c=kt*128+p]
    opT = np.ascontiguousarray(
        out_proj_w.reshape(2, 512, KC, 128).transpose(0, 2, 3, 1)).astype(bf16)
    convw_r = np.ascontiguousarray(
        conv_w.reshape(KC, 128, 4).transpose(1, 0, 2).reshape(128, KC * 4))
    convb_r = np.ascontiguousarray(conv_b.reshape(KC, 128).T)
    gateb_r = np.ascontiguousarray(gate_b.reshape(KC, 128).T)
    cx_r = np.ascontiguousarray(cbias[:DI].reshape(KC, 128).T)
    cz_r = np.ascontiguousarray(cbias[DI:].reshape(KC, 128).T)

    in_maps = []
    for core in range(8):
        b, j = core // 4, core % 4
        xs = np.zeros((NT, D), np.float32)
        start = j * CHUNK - W
        mask = np.ones((1, W), np.float32)
        if j == 0:
            xs[W:] = x[b, 0:CHUNK]
            mask[0, :W] = 0.0
        else:
            xs[:] = x[b, start:start + NT]
        in_maps.append({
            "x": np.ascontiguousarray(xs), "w1x": w1xT, "w1z": w1zT,
            "gw8": gw8, "opw": opT, "convw": convw_r, "convb": convb_r,
            "gateb": gateb_r, "cx": cx_r, "cz": cz_r,
            "mask": mask.astype(bf16),
        })
    return in_maps


def kernel(x, norm_w, norm_b, in_proj_w, conv_w, conv_b, gate_w, gate_b,
           out_proj_w, _trace=False, _collect=None):
    x = np.asarray(x, np.float32)
    if "nc" not in _cache:
        _cache["nc"] = _build()
    nc = _cache["nc"]
    in_maps = _prep_host(
        x, np.asarray(norm_w, np.float32), np.asarray(norm_b, np.float32),
        np.asarray(in_proj_w, np.float32), np.asarray(conv_w, np.float32),
        np.asarray(conv_b, np.float32), np.asarray(gate_w, np.float32),
        np.asarray(gate_b, np.float32), np.asarray(out_proj_w, np.float32))
    res = run_bass_kernel_spmd(nc, in_maps, core_ids=list(range(8)), trace=_trace)
    if _collect is not None:
        _collect.append(res)
    out = np.empty((B, L, D), np.float32)
    for core in range(8):
        b, j = core // 4, core % 4
        out[b, j * CHUNK:(j + 1) * CHUNK] = res.results[core]["out"]
    return out


# revision 21
# speedup vs baseline: 1.8946x; 1.0012x over previous
"""GatedLinearRecurrence Trainium2 kernel (8-core SPMD, Bass/Tile).

Strategy: shard (batch=2) x (4 sequence chunks of 1024 tokens) across 8 cores.
Each core processes 1152 tokens: a 128-token warm-up window (recomputed
redundantly; the recurrence decay makes carry-in truncation error ~1e-24)
followed by its 1024 "main" tokens.  No collectives needed.

v2 changes vs baseline:
  * bf16 operands for in_proj / out_proj / transposes (same PE rate as f32r
    at these widths, but half the DMA + SBUF, 1.0 c/row transposes).
  * gate matmul in fp8 e4m3 with MatmulPerfMode.DoubleRow (K=256 per
    instruction, 2x PE throughput).  gw is pre-scaled x32 and xc x4 on the
    fp8 copy; the 1/128 is folded into the sigmoid evacuation scale.
    Simulated end-to-end rel err 0.0049 (gate 2e-2).
  * norm_b folded into a per-output-channel in_proj bias (host-computed
    c = in_proj_w @ norm_b), so transposed x-hat needs no bias and all 8
    d-tiles of a token tile evacuate PSUM in ONE scalar op.
  * z kept in SBUF as silu(z) bf16 (no HBM scratch roundtrip).
  * phase order S2(in_proj-x) -> S3'(gate et interleaved with z et) -> S7
    (out_proj): the PE never waits on the DVE scans, z fills the gaps.
  * scans/bt/yz alternate vector/gpsimd by et parity; conv split across
    vector/gpsimd; out_proj final pass runs kt-inner so the 8 tail
    evacuations stagger instead of serializing.
"""
import sys

for p in ("/opt/trn_rl_repo", "/root/.axon_site/_ro/trn_rl_repo"):
    if p not in sys.path:
        sys.path.insert(0, p)

import numpy as np
import ml_dtypes

import concourse.bass as bass
import concourse.bacc as bacc
import concourse.tile as tile
import concourse.mybir as mybir
from concourse.bass_utils import run_bass_kernel_spmd
from concourse.masks import make_identity

F32 = mybir.dt.float32
BF16 = mybir.dt.bfloat16
FP8 = mybir.dt.float8e4
AF = mybir.ActivationFunctionType
OP = mybir.AluOpType
DR = mybir.MatmulPerfMode.DoubleRow

B, L, D = 2, 4096, 1024
DI = 2048            # d_inner
NT = 1152            # tokens per core (128 warm-up + 1024 main)
W = 128              # warm-up tokens
CHUNK = 1024
NTT = NT // 128      # 9 token tiles
KD = D // 128        # 8 k-tiles over d_model
KC = DI // 128       # 16 k-tiles over d_inner
KC2 = KC // 2        # 8 fp8 k-pair tiles (DoubleRow contracts 256)
TC = 384             # matmul N chunk (3 per core)
NTC = NT // TC
EPS = 1e-5
SW = 32.0            # gate weight fp8 pre-scale
SX = 4.0             # xc fp8 pre-scale

_cache = {}


def _build():
    nc = bacc.Bacc(None, target_bir_lowering=False)

    x_h = nc.dram_tensor("x", [NT, D], F32, kind="ExternalInput")
    w1x_h = nc.dram_tensor("w1x", [KC, 128, KD * 128], BF16, kind="ExternalInput")
    w1z_h = nc.dram_tensor("w1z", [KC, 128, KD * 128], BF16, kind="ExternalInput")
    gw8_h = nc.dram_tensor("gw8", [KC, 128, KC2 * 2 * 128], FP8, kind="ExternalInput")
    op_h = nc.dram_tensor("opw", [2, KC, 128, 512], BF16, kind="ExternalInput")
    convw_h = nc.dram_tensor("convw", [128, KC * 4], F32, kind="ExternalInput")
    convb_h = nc.dram_tensor("convb", [128, KC], F32, kind="ExternalInput")
    gateb_h = nc.dram_tensor("gateb", [128, KC], F32, kind="ExternalInput")
    cx_h = nc.dram_tensor("cx", [128, KC], F32, kind="ExternalInput")
    cz_h = nc.dram_tensor("cz", [128, KC], F32, kind="ExternalInput")
    mask_h = nc.dram_tensor("mask", [1, W], BF16, kind="ExternalInput")
    out_h = nc.dram_tensor("out", [CHUNK, D], F32, kind="ExternalOutput")

    with tile.TileContext(nc) as tc:
        with tc.tile_pool(name="consts", bufs=1) as consts:

            ident = consts.tile([128, 128], BF16, name="ident")
            make_identity(nc, ident)
            mask_sb = consts.tile([128, W], BF16, name="mask_sb")
            nc.gpsimd.dma_start(
                out=mask_sb,
                in_=bass.AP(tensor=mask_h, offset=0, ap=[[0, 128], [1, W]]))
            convw = consts.tile([128, KC * 4], F32, name="convw")
            nc.gpsimd.dma_start(out=convw, in_=convw_h.ap())
            convb = consts.tile([128, KC], F32, name="convb")
            nc.gpsimd.dma_start(out=convb, in_=convb_h.ap())
            gateb = consts.tile([128, KC], F32, name="gateb")
            nc.gpsimd.dma_start(out=gateb, in_=gateb_h.ap())
            cxb = consts.tile([128, KC], F32, name="cxb")
            nc.gpsimd.dma_start(out=cxb, in_=cx_h.ap())
            czb = consts.tile([128, KC], F32, name="czb")
            nc.gpsimd.dma_start(out=czb, in_=cz_h.ap())
            eps_t = consts.tile([128, 1], F32, name="eps_t")
            nc.vector.memset(eps_t, EPS)

            # long-lived activation stores
            with tc.tile_pool(name="xcp", bufs=1) as xcp, \
                 tc.tile_pool(name="xc8p", bufs=1) as xc8p, \
                 tc.tile_pool(name="gws", bufs=4) as gs, \
                 tc.tile_pool(name="wz3", bufs=3) as wzs, \
                 tc.tile_pool(name="zsil", bufs=4) as zsp, \
                 tc.tile_pool(name="xT", bufs=1) as xTp:

                xc = [xcp.tile([128, NT], BF16, name=f"xct{e}") for e in range(KC)]
                xc8 = [xc8p.tile([128, 2, NT], FP8, name=f"xc8t{j}")
                       for j in range(KC2)]
                # x-hat-T per chunk: [128 d-part, KD d-tiles, TC tokens]
                xT = [xTp.tile([128, KD, TC], BF16, name=f"xTt{c_}")
                      for c_ in range(NTC)]

                gts = {}
                wzts = {}

                # ---- S1-S2: LN, transpose, in_proj x-half, conv, silu ----
                with tc.tile_pool(name="s1roll", bufs=2) as s1r, \
                     tc.tile_pool(name="stat", bufs=4) as stp, \
                     tc.tile_pool(name="w1s", bufs=3) as ws, \
                     tc.tile_pool(name="psmm", bufs=4, space="PSUM") as psmm, \
                     tc.tile_pool(name="pstr", bufs=3, space="PSUM") as pstr:

                    for it in range(NTT):
                        tc3, col = it // 3, (it % 3) * 128
                        xt = s1r.tile([128, D], F32, tag="xt", bufs=4, name="xt")
                        if it < 3:
                            qs = (nc.sync, nc.scalar, nc.gpsimd, nc.sync)
                            for q_ in range(4):
                                qs[q_].dma_start(
                                    out=xt[:, q_ * 256:(q_ + 1) * 256],
                                    in_=x_h.ap()[it * 128:(it + 1) * 128,
                                                 q_ * 256:(q_ + 1) * 256])
                        else:
                            qs = (nc.sync, nc.scalar, nc.gpsimd)
                            qs[it % 3].dma_start(
                                out=xt[:, 0:512],
                                in_=x_h.ap()[it * 128:(it + 1) * 128, 0:512])
                            qs[(it + 1) % 3].dma_start(
                                out=xt[:, 512:1024],
                                in_=x_h.ap()[it * 128:(it + 1) * 128, 512:1024])
                        if it == 2:
                            # prefetch gate + z weights once the head DMAs are out
                            for et_ in range(2):
                                gt = gs.tile([128, KC2, 2, 128], FP8, tag="gw",
                                             name=f"gt{et_}")
                                nc.gpsimd.dma_start(out=gt, in_=gw8_h.ap()[et_])
                                gts[et_] = gt
                            for et_ in range(3):
                                wz = wzs.tile([128, KD, 128], BF16, tag="wz",
                                              name="wtz")
                                nc.sync.dma_start(out=wz, in_=w1z_h.ap()[et_])
                                wzts[et_] = wz
                        stats = stp.tile([128, 2, 6], F32, tag="stats", name="stats")
                        nc.vector.bn_stats(out=stats[:, 0, :], in_=xt[:, 0:512])
                        nc.vector.bn_stats(out=stats[:, 1, :], in_=xt[:, 512:1024])
                        mv = stp.tile([128, 2], F32, tag="mv", name="mv")
                        nc.vector.bn_aggr(out=mv, in_=stats)
                        rstd = stp.tile([128, 1], F32, tag="rstd", name="rstd")
                        nc.scalar.activation(out=rstd, in_=mv[:, 1:2], func=AF.Sqrt,
                                             bias=eps_t, scale=1.0)
                        nc.vector.reciprocal(out=rstd, in_=rstd)
                        nmr = stp.tile([128, 1], F32, tag="nmr", name="nmr")
                        nc.vector.tensor_scalar(out=nmr, in0=mv[:, 0:1],
                                                scalar1=rstd, scalar2=-1.0,
                                                op0=OP.mult, op1=OP.mult)
                        xh = s1r.tile([128, D], BF16, tag="xh", bufs=3, name="xh")
                        # LN apply on the Scalar engine: x*rstd - mu*rstd
                        nc.scalar.activation(out=xh, in_=xt, func=AF.Identity,
                                             scale=rstd, bias=nmr)
                        pst = pstr.tile([128, KD, 128], BF16, tag="tr", name="pst")
                        # one accumulation group over the 8 disjoint d-tile
                        # regions: first write after start zeroes per-byte
                        for d_ in range(KD):
                            nc.tensor.matmul(pst[:, d_, :],
                                             xh[:, d_ * 128:(d_ + 1) * 128], ident,
                                             start=(d_ == 0), stop=(d_ == KD - 1),
                                             is_transpose=True,
                                             skip_group_check=True)
                        # one evacuation for all 8 d-tiles of this token tile
                        nc.scalar.activation(
                            out=xT[tc3][:, :, col:col + 128], in_=pst,
                            func=AF.Identity, scale=1.0)

                    # in_proj x-half + conv + silu + warm-up mask + fp8 cast.
                    NW = 6
                    order = [(e, c) for c in range(NTC) for e in range(NW)]
                    order += [(e, c) for e in range(NW, KC) for c in range(NTC)]
                    wts, xins = {}, {}

                    def s2_chain(et, tc3):
                        if tc3 == 0:
                            wt = ws.tile([128, KD, 128], BF16, tag="w1",
                                         bufs=6, name=f"wt{et}")
                            nc.sync.dma_start(out=wt, in_=w1x_h.ap()[et])
                            wts[et] = wt
                            xin = s1r.tile([128, NT + 3], BF16, tag="xin",
                                           bufs=NW + 2, name=f"xin{et}")
                            nc.vector.memset(xin[:, 0:3], 0.0)
                            xins[et] = xin
                        ps = psmm.tile([128, TC], F32, tag="mm", name="ps")
                        for kt in range(KD):
                            nc.tensor.matmul(
                                ps, wts[et][:, kt, :], xT[tc3][:, kt, :],
                                start=(kt == 0), stop=(kt == KD - 1))
                        nc.scalar.activation(
                            out=xins[et][:, 3 + tc3 * TC: 3 + (tc3 + 1) * TC],
                            in_=ps, func=AF.Identity, bias=cxb[:, et:et + 1],
                            scale=1.0)
                        if tc3 == NTC - 1:
                            xin = xins.pop(et)
                            tmp = s1r.tile([128, NT], BF16, tag="ctmp", bufs=3,
                                           name="ctmp")
                            nc.vector.tensor_scalar_mul(
                                tmp, xin[:, 0:NT], convw[:, et * 4:et * 4 + 1])
                            for k in range(1, 4):
                                nc.vector.scalar_tensor_tensor(
                                    out=tmp, in0=xin[:, k:k + NT],
                                    scalar=convw[:, et * 4 + k:et * 4 + k + 1],
                                    in1=tmp, op0=OP.mult, op1=OP.add)
                            nc.scalar.activation(out=xc[et], in_=tmp, func=AF.Silu,
                                                 bias=convb[:, et:et + 1], scale=1.0)
                            # mask is non-unit only on the warm-up columns
                            nc.vector.tensor_mul(
                                xc[et][:, 0:W], xc[et][:, 0:W], mask_sb)
                            # fp8 copy (x SX) for the gate matmul rhs
                            nc.scalar.activation(
                                out=xc8[et // 2][:, et % 2, :], in_=xc[et],
                                func=AF.Copy, scale=SX)

                    for et, tc3 in order:
                        s2_chain(et, tc3)

                # ---- S3': per et: gate (fp8 DoubleRow) + z-half (bf16),
                # sigmoid/scan/yz trail on scalar + vector/gpsimd. ----
                with tc.tile_pool(name="yp", bufs=1) as yp, \
                     tc.tile_pool(name="ops", bufs=18) as opp, \
                     tc.tile_pool(name="s7res", bufs=16) as s7x:
                    ych = [None] * KC
                    NTB = CHUNK // 128
                    opts0, xres0 = [], []
                    with tc.tile_pool(name="ach", bufs=3) as ayp, \
                         tc.tile_pool(name="btr", bufs=2) as btp, \
                         tc.tile_pool(name="sgr", bufs=4) as sgp, \
                         tc.tile_pool(name="psg", bufs=2, space="PSUM") as psg, \
                         tc.tile_pool(name="psz", bufs=3, space="PSUM") as psz:

                        zs = {}

                        def z_half(et):
                            wt = wzts.pop(et)
                            zt = zsp.tile([128, CHUNK], BF16, tag="zs", name="zst")
                            zs[et] = zt
                            for tc3 in range(NTC):
                                lo = W if tc3 == 0 else tc3 * TC   # NT-space
                                n = (tc3 + 1) * TC - lo
                                ps = psz.tile([128, TC], F32, tag="zmm", name="pszt")
                                for kt in range(KD):
                                    nc.tensor.matmul(
                                        ps[:, 0:n], wt[:, kt, :],
                                        xT[tc3][:, kt, TC - n:TC],
                                        start=(kt == 0), stop=(kt == KD - 1))
                                # silu(v) = v*sigmoid(v): scalar stays on the
                                # Sigmoid table all phase, vector fuses the
                                # add+mult straight from PSUM
                                sg = sgp.tile([128, TC], BF16, tag="sg", name="sg")
                                nc.scalar.activation(
                                    out=sg[:, 0:n], in_=ps[:, 0:n], func=AF.Sigmoid,
                                    bias=czb[:, et:et + 1], scale=1.0)
                                nc.vector.scalar_tensor_tensor(
                                    out=zt[:, lo - W:(tc3 + 1) * TC - W],
                                    in0=ps[:, 0:n], scalar=czb[:, et:et + 1],
                                    in1=sg[:, 0:n], op0=OP.add, op1=OP.mult)

                        def gate(et):
                            gt = gts.pop(et)
                            if et + 2 < KC:
                                g2 = gs.tile([128, KC2, 2, 128], FP8, tag="gw",
                                             name=f"gt{et + 2}")
                                nc.gpsimd.dma_start(out=g2, in_=gw8_h.ap()[et + 2])
                                gts[et + 2] = g2
                            if et + 3 < KC:
                                wz = wzs.tile([128, KD, 128], BF16, tag="wz",
                                              name="wtz")
                                nc.sync.dma_start(out=wz, in_=w1z_h.ap()[et + 3])
                                wzts[et + 3] = wz
                            # prefetch out_proj nb=0 weights + residual rows
                            opt = opp.tile([128, 512], BF16, tag="opw", name="opt")
                            nc.gpsimd.dma_start(out=opt, in_=op_h.ap()[0, et])
                            opts0.append(opt)
                            if et >= 8:
                                tb = et - 8
                                xr = s7x.tile([128, 512], F32, tag="xres",
                                              name=f"xres{tb}")
                                nc.scalar.dma_start(
                                    out=xr,
                                    in_=x_h.ap()[W + tb * 128:W + (tb + 1) * 128,
                                                 0:512])
                                xres0.append(xr)
                            a_t = ayp.tile([128, NT], BF16, tag="ach", name="ach")
                            for tc3 in range(NTC):
                                ps = psg.tile([128, TC], F32, tag="gmm", name="psgt")
                                for j in range(KC2):
                                    nc.tensor.matmul(
                                        ps, gt[:, j], xc8[j][:, :, tc3 * TC:(tc3 + 1) * TC],
                                        start=(j == 0), stop=(j == KC2 - 1),
                                        perf_mode=DR)
                                nc.scalar.activation(
                                    out=a_t[:, tc3 * TC:(tc3 + 1) * TC], in_=ps,
                                    func=AF.Sigmoid,
                                    bias=gateb[:, et:et + 1], scale=1.0 / (SW * SX))
                            # full-width bt/scan/yz: fewer DVE ops, no chaining
                            bt = btp.tile([128, NT], BF16, tag="bt", name="bt")
                            nc.vector.scalar_tensor_tensor(
                                out=bt, in0=a_t, scalar=1.0, in1=xc[et],
                                op0=OP.subtract, op1=OP.mult)
                            y_t = yp.tile([128, NT], BF16, name=f"y{et}")
                            nc.vector.tensor_tensor_scan(
                                out=y_t, data0=a_t, data1=bt, initial=0.0,
                                op0=OP.mult, op1=OP.add)
                            zt = zs.pop(et)
                            nc.vector.tensor_mul(y_t[:, W:NT], y_t[:, W:NT], zt)
                            ych[et] = y_t

                        z_half(0)
                        z_half(1)
                        for et in range(KC):
                            gate(et)
                            if et + 2 < KC:
                                z_half(et + 2)

                    # ---- S7: out_proj + residual. ----

                    def yslice(kt, tb):
                        col = W + tb * 128          # absolute column in [0, NT)
                        return ych[kt][:, col:col + 128]

                    with tc.tile_pool(name="s7roll", bufs=6) as s7r, \
                         tc.tile_pool(name="psop", bufs=8, space="PSUM") as psop:
                        for nb in range(2):
                            if nb == 0:
                                xres = xres0
                                opts = opts0
                            else:
                                xres = []
                                for tb in range(NTB):
                                    xr = s7x.tile([128, 512], F32, tag="xres",
                                                  name=f"xres{tb}")
                                    nc.scalar.dma_start(
                                        out=xr,
                                        in_=x_h.ap()[W + tb * 128:W + (tb + 1) * 128,
                                                     512:1024])
                                    xres.append(xr)
                                opts = []
                                for kt in range(KC):
                                    opt = opp.tile([128, 512], BF16, tag="opw",
                                                   name="opt")
                                    nc.gpsimd.dma_start(out=opt,
                                                        in_=op_h.ap()[1, kt])
                                    opts.append(opt)
                            pss = [psop.tile([128, 512], F32, tag="op",
                                             name=f"pso{tb}") for tb in range(NTB)]
                            if nb == 0:
                                # kt-outer: max stationary reuse
                                for kt in range(KC):
                                    for tb in range(NTB):
                                        nc.tensor.matmul(
                                            pss[tb], yslice(kt, tb), opts[kt],
                                            start=(kt == 0), stop=(kt == KC - 1))
                            else:
                                # kt-inner: stagger the final evacuations
                                for tb in range(NTB):
                                    for kt in range(KC):
                                        nc.tensor.matmul(
                                            pss[tb], yslice(kt, tb), opts[kt],
                                            start=(kt == 0), stop=(kt == KC - 1))
                            for tb in range(NTB):
                                oh = s7r.tile([128, 512], F32, tag="oh", name="oh")
                                nc.vector.tensor_sub(oh, xres[tb], pss[tb])
                                deng = nc.sync if tb % 2 == 0 else nc.scalar
                                deng.dma_start(
                                    out=out_h.ap()[tb * 128:(tb + 1) * 128,
                                                   nb * 512:(nb + 1) * 512],
                                    in_=oh)

    nc.compile()
    return nc


def _prep_host(x, norm_w, norm_b, in_proj_w, conv_w, conv_b, gate_w, gate_b,
               out_proj_w):
    bf16 = ml_dtypes.bfloat16
    fp8 = ml_dtypes.float8_e4m3
    w1f = (in_proj_w * norm_w[None, :]).astype(np.float32)
    cbias = (in_proj_w.astype(np.float64) @ norm_b.astype(np.float64)).astype(np.float32)
    # w1 tiles pre-arranged [et, p, kt*128+e]: w1r[et, p, kt, e] = w1f[e_out=et*128+e, d=kt*128+p]
    def w1_tiles(wh):                                          # wh [DI, D]
        t = wh.reshape(KC, 128, KD, 128).transpose(0, 3, 2, 1)  # et, p, kt, e
        return np.ascontiguousarray(t.reshape(KC, 128, KD * 128)).astype(bf16)
    w1xT = w1_tiles(w1f[:DI])
    w1zT = w1_tiles(w1f[DI:])
    # gw8[et, p, j, s, m] = gw[et*128+m, j*256+s*128+p] * SW
    gwT = np.ascontiguousarray(gate_w.T * SW)                 # [c_in, e_out]
    gw8 = gwT.reshape(KC2, 2, 128, KC, 128).transpose(3, 2, 0, 1, 4)
    gw8 = np.ascontiguousarray(gw8.reshape(KC, 128, KC2 * 2 * 128)).astype(fp8)
    # opw tiles [nb, kt, p, e] = out_proj_w[e_out=nb*512+e, c=kt*128+p]
    opT = np.ascontiguousarray(
        out_proj_w.reshape(2, 512, KC, 128).transpose(0, 2, 3, 1)).astype(bf16)
    convw_r = np.ascontiguousarray(
        conv_w.reshape(KC, 128, 4).transpose(1, 0, 2).reshape(128, KC * 4))
    convb_r = np.ascontiguousarray(conv_b.reshape(KC, 128).T)
    gateb_r = np.ascontiguousarray(gate_b.reshape(KC, 128).T)
    cx_r = np.ascontiguousarray(cbias[:DI].reshape(KC, 128).T)
    cz_r = np.ascontiguousarray(cbias[DI:].reshape(KC, 128).T)

    in_maps = []
    for core in range(8):
        b, j = core // 4, core % 4
        xs = np.zeros((NT, D), np.float32)
        start = j * CHUNK - W
        mask = np.ones((1, W), np.float32)
        if j == 0:
            xs[W:] = x[b, 0:CHUNK]
            mask[0, :W] = 0.0
        else:
            xs[:] = x[b, start:start + NT]
        in_maps.append({
            "x": np.ascontiguousarray(xs), "w1x": w1xT, "w1z": w1zT,
            "gw8": gw8, "opw": opT, "convw": convw_r, "convb": convb_r,
            "gateb": gateb_r, "cx": cx_r, "cz": cz_r,
            "mask": mask.astype(bf16),
        })
    return in_maps


def kernel(x, norm_w, norm_b, in_proj_w, conv_w, conv_b, gate_w, gate_b,
           out_proj_w, _trace=False, _collect=None):
    x = np.asarray(x, np.float32)
    if "nc" not in _cache:
        _cache["nc"] = _build()
    nc = _cache["nc"]
    in_maps = _prep_host(
        x, np.asarray(norm_w, np.float32), np.asarray(norm_b, np.float32),
        np.asarray(in_proj_w, np.float32), np.asarray(conv_w, np.float32),
        np.asarray(conv_b, np.float32), np.asarray(gate_w, np.float32),
        np.asarray(gate_b, np.float32), np.asarray(out_proj_w, np.float32))
    res = run_bass_kernel_spmd(nc, in_maps, core_ids=list(range(8)), trace=_trace)
    if _collect is not None:
        _collect.append(res)
    out = np.empty((B, L, D), np.float32)
    for core in range(8):
        b, j = core // 4, core % 4
        out[b, j * CHUNK:(j + 1) * CHUNK] = res.results[core]["out"]
    return out
